# revision 17
# baseline (speedup 1.0000x reference)
"""Trainium2 Bass kernel for nn_AdaptiveModelV3 (LSTM + episodic memory read).

Strategy (hardcoded, per sharding hint): data-parallel over batch on 8
NeuronCores; the tiny memory bank (200x32 keys/vals from sample 0) is
computed host-side and replicated to every core.  No collectives.

Per-core device kernel:
  - LSTM scan over 201 steps in "gate-transposed" layout: partitions =
    128 hidden units of one gate, free dim = batch.  h_t comes out of
    the pointwise stage already transposed for the next step's matmul.
  - Biases are folded into the matmul via a ones-row appended to x
    (contraction K = 65 for the input projection).
  - Gate order in PSUM is [i, f, o, g]: one merged Sigmoid over [i,f,o]
    (FD=768) + one Tanh over g per step per stream.
  - Batch 512/core is split into 2 streams of 256, pipelined so the
    serial pointwise chain of one stream hides under the other's work.
  - Attention read (cosine sim softmax over 200 slots) + output head in
    fp32/bf16, a few microseconds total.
"""

import sys

import numpy as np

try:
    import concourse.bass as bass
except ImportError:  # pragma: no cover
    sys.path.insert(0, "/opt/trn_rl_repo")
    import concourse.bass as bass

import ml_dtypes
from contextlib import ExitStack

import concourse.mybir as mybir
import concourse.tile as tile
from concourse import bacc
from concourse.bass_utils import run_bass_kernel_spmd

BF16 = ml_dtypes.bfloat16

# Problem constants (hardcoded per spec).
B, S, DIN = 4096, 201, 64
H = 128
KD, DK, DV = 32, 32, 32
DOUT = 32
TEMP = 0.1
NOVELTY = 0.5
EPS = 1e-8
NSLOTS = S - 1  # 200

NCORES = 8
BC = B // NCORES  # 512 batch per core
NSTREAM = 2
BS = BC // NSTREAM  # 256 batch per stream
TC = 67  # time chunk (201 = 3 * 67)
NCHUNK = S // TC

# Gate order in reference (PyTorch): [i, f, g, o].  Our PSUM layout is
# [i, f, o, g] so sigmoid covers a contiguous [i,f,o] block.
GATE_SRC = [0, 1, 3, 2]
D1_FILL = 32   # PE filler mms after rc (covers the sigma wait)
D2_FILL = 0    # PE filler mms between ic and rc (covers the h wait)
F32 = mybir.dt.float32
BF = mybir.dt.bfloat16
AF = mybir.ActivationFunctionType
ALU = mybir.AluOpType


def build_kernel(s_steps=S, no_rc=False):
    nc = bacc.Bacc()

    # ---- DRAM parameters (per-core shapes; all cores run the same graph)
    xaug = nc.dram_tensor("xaug", [DIN + 1, S, BC], BF, kind="ExternalInput")
    xq = nc.dram_tensor("xq", [KD + 1, BC], F32, kind="ExternalInput")
    wih = nc.dram_tensor("wih", [DIN + 1, 4, H], BF, kind="ExternalInput")
    whh = nc.dram_tensor("whh", [H, 4, H], BF, kind="ExternalInput")
    wk = nc.dram_tensor("wk", [KD + 1, DK], F32, kind="ExternalInput")
    knt = nc.dram_tensor("knt", [DK, NSLOTS], F32, kind="ExternalInput")
    maskb = nc.dram_tensor("maskb", [NSLOTS, 1], F32, kind="ExternalInput")
    vv = nc.dram_tensor("vv", [NSLOTS, DV], F32, kind="ExternalInput")
    woh = nc.dram_tensor("woh", [H, DOUT], BF, kind="ExternalInput")
    woc = nc.dram_tensor("woc", [DV, DOUT], BF, kind="ExternalInput")
    bo = nc.dram_tensor("bo", [DOUT, 1], F32, kind="ExternalInput")
    out = nc.dram_tensor("out", [DOUT, BC], F32, kind="ExternalOutput")
    dbg_h = nc.dram_tensor("dbg_h", [H, BC], F32, kind="ExternalOutput")
    dbg_c = nc.dram_tensor("dbg_c", [H, BC], F32, kind="ExternalOutput")
    dbg_ctx = nc.dram_tensor("dbg_ctx", [DV, BC], F32, kind="ExternalOutput")
    dbg_qn = nc.dram_tensor("dbg_qn", [DK, BC], F32, kind="ExternalOutput")

    with tile.TileContext(nc) as tc, ExitStack() as ctx:
        consts = ctx.enter_context(tc.tile_pool(name="consts", bufs=1))
        xpool = ctx.enter_context(tc.tile_pool(name="xp", bufs=2))
        work = ctx.enter_context(tc.tile_pool(name="work", bufs=2))

        # ---- weights / consts to SBUF
        sb_wih = consts.tile([DIN + 1, 4, H], BF, tag="wih")
        nc.sync.dma_start(sb_wih[:], wih[:])
        sb_whh = consts.tile([H, 4, H], BF, tag="whh")
        nc.sync.dma_start(sb_whh[:], whh[:])
        sb_xq = consts.tile([KD + 1, BC], F32, tag="xq")
        nc.sync.dma_start(sb_xq[:], xq[:])
        sb_wk = consts.tile([KD + 1, DK], F32, tag="wk")
        nc.sync.dma_start(sb_wk[:], wk[:])
        sb_knt = consts.tile([DK, NSLOTS], F32, tag="knt")
        nc.sync.dma_start(sb_knt[:], knt[:])
        sb_mb1 = consts.tile([128, 1], F32, tag="mb1")
        nc.sync.dma_start(sb_mb1[:], maskb[0:128])
        sb_mb2 = consts.tile([NSLOTS - 128, 1], F32, tag="mb2")
        nc.sync.dma_start(sb_mb2[:], maskb[128:NSLOTS])
        sb_v1 = consts.tile([128, DV], F32, tag="v1")
        nc.sync.dma_start(sb_v1[:], vv[0:128])
        sb_v2 = consts.tile([NSLOTS - 128, DV], F32, tag="v2")
        nc.sync.dma_start(sb_v2[:], vv[128:NSLOTS])
        sb_woh = consts.tile([H, DOUT], BF, tag="woh")
        nc.sync.dma_start(sb_woh[:], woh[:])
        sb_woc = consts.tile([DV, DOUT], BF, tag="woc")
        nc.sync.dma_start(sb_woc[:], woc[:])
        sb_bo = consts.tile([DOUT, 1], F32, tag="bo")
        nc.sync.dma_start(sb_bo[:], bo[:])

        ones_q = consts.tile([DK, DK], F32, tag="ones_q")
        nc.vector.memset(ones_q[:], 1.0)
        ones_s1 = consts.tile([128, DK], F32, tag="ones_s1")
        nc.vector.memset(ones_s1[:], 1.0)
        ones_s2 = consts.tile([NSLOTS - 128, DK], F32, tag="ones_s2")
        nc.vector.memset(ones_s2[:], 1.0)

        # ================= attention read (independent of the LSTM) ====
        psA_cm = tc.tile_pool(name="psA", bufs=1, space="PSUM")
        psA = psA_cm.__enter__()
        # qT = Wk @ [x_q; 1]  -> [DK, BC] in PSUM
        q_ps = psA.tile([DK, BC], F32, tag="q")
        nc.tensor.matmul(q_ps[:], sb_wk[:], sb_xq[:], start=True, stop=True)
        # copy to SBUF (Copy is in every ACT table set)
        qc = consts.tile([DK, BC], F32, tag="qc")
        nc.scalar.copy(qc[:], q_ps[:])
        # squared + column sums (replicated to all DK partitions via ones)
        q2 = consts.tile([DK, BC], F32, tag="q2")
        nc.vector.tensor_tensor(q2[:], qc[:], qc[:], ALU.mult)
        n_ps = psA.tile([DK, BC], F32, tag="n")
        nc.tensor.matmul(n_ps[:], ones_q[:], q2[:], start=True, stop=True)
        # s = n^(-1/2) = exp(-0.5 * ln(n))   (||q|| >> eps, so eps ignored)
        lnn = consts.tile([DK, BC], F32, tag="lnn")
        nc.scalar.activation(lnn[:], n_ps[:], AF.Ln)
        s32 = consts.tile([DK, BC], F32, tag="s32")
        nc.scalar.activation(s32[:], lnn[:], AF.Exp, scale=-0.5)
        qn = consts.tile([DK, BC], F32, tag="qn")
        nc.vector.tensor_tensor(qn[:], qc[:], s32[:], ALU.mult)

        # logits chunks: L = kn @ qn  ([slots, BC])
        l1_ps = psA.tile([128, BC], F32, tag="l1")
        nc.tensor.matmul(l1_ps[:], sb_knt[:, 0:128], qn[:], start=True, stop=True)
        l2_ps = psA.tile([NSLOTS - 128, BC], F32, tag="l2")
        nc.tensor.matmul(l2_ps[:], sb_knt[:, 128:NSLOTS], qn[:], start=True, stop=True)
        e1 = consts.tile([128, BC], F32, tag="e1")
        nc.scalar.activation(e1[:], l1_ps[:], AF.Exp, bias=sb_mb1[:], scale=1.0 / TEMP)
        e2 = consts.tile([NSLOTS - 128, BC], F32, tag="e2")
        nc.scalar.activation(e2[:], l2_ps[:], AF.Exp, bias=sb_mb2[:], scale=1.0 / TEMP)

        # denominator, replicated over DK partitions; r = 1/S
        s_ps = psA.tile([DK, BC], F32, tag="s")
        nc.tensor.matmul(s_ps[:], ones_s1[:], e1[:], start=True, stop=False)
        nc.tensor.matmul(s_ps[:], ones_s2[:], e2[:], start=False, stop=True)
        lns = consts.tile([DK, BC], F32, tag="lns")
        nc.scalar.activation(lns[:], s_ps[:], AF.Ln)
        rs = consts.tile([DK, BC], F32, tag="rs")
        nc.scalar.activation(rs[:], lns[:], AF.Exp, scale=-1.0)

        # ctxT = V^T @ E  -> [DV, BC]; then normalize and cast to bf16
        ctx_ps = psA.tile([DV, BC], F32, tag="c")
        nc.tensor.matmul(ctx_ps[:], sb_v1[:], e1[:], start=True, stop=False)
        nc.tensor.matmul(ctx_ps[:], sb_v2[:], e2[:], start=False, stop=True)
        ctxn = consts.tile([DV, BC], F32, tag="ctxn")
        nc.vector.tensor_tensor(ctxn[:], ctx_ps[:], rs[:], ALU.mult)
        ctxb = consts.tile([DV, BC], BF, tag="ctxb")
        nc.vector.tensor_copy(ctxb[:], ctxn[:])

        psA_cm.__exit__(None, None, None)

        # ========================= LSTM scan ===========================
        psL_cm = tc.tile_pool(name="psL", bufs=1, space="PSUM")
        psL = psL_cm.__enter__()
        h_tiles = [None, None]
        c_tiles = [None, None]
        xc_tiles = {}

        for ci in range(NCHUNK):
            xc = xpool.tile([DIN + 1, TC, BC], BF, tag="xc")
            nc.sync.dma_start(xc[:], xaug[:, ci * TC : (ci + 1) * TC, :])
            xc_tiles[ci] = xc

        _dummy_rhs = sb_whh[:, 1:3].rearrange("k a b -> k (a b)")
        # psum per stream: [128, 4, 512] fp32, ONE BANK PER GATE (cols
        # 0:BS used).  start=True clears has_written at bank granularity,
        # so bank isolation lets ic mms prefetch in any order before the
        # h-dependent rc mms.  bufs=1: 4 banks x 2 streams = all of PSUM;
        # the only psum reader is the merged sigmoid, so step t+1's ic
        # mms just wait on sigma(t).
        for t in range(s_steps):
            xc = xc_tiles[t // TC]
            ti = t % TC
            ps_t = []
            for s in range(NSTREAM):
                ps = psL.tile([128, 4, 512], F32, tag=f"lstm{s}")
                ps_t.append(ps)
            # input-projection mms: prefetchable (no h dependency)
            for g in range(4):
                for s in range(NSTREAM):
                    xs = xc[:, ti, s * BS : (s + 1) * BS]
                    nc.tensor.matmul(
                        ps_t[s][:, g, 0:BS], sb_wih[:, g], xs,
                        start=True, stop=(t == 0),
                    )
            if t == 0:
                # PE warm-up: ~48 back-to-back matmuls into the unused
                # scratch half of the step-0 psum banks.  One contiguous
                # >3.4us burst flips the HAM clock gate to 2.4GHz; the
                # per-step gaps afterwards are too short to re-throttle.
                for w in range(48):
                    nc.tensor.matmul(
                        ps_t[0][:, 0, BS : BS + 128], sb_whh[:, 1],
                        sb_whh[:, 2], start=False, stop=False,
                        skip_group_check=True,
                    )
            # PE filler: keep duty near 100% so HAM holds the 2.4GHz clock.
            # Dummies write the unused scratch half of the step's psum banks.
            def dummy(n, tile_idx=0):
                for _ in range(n):
                    nc.tensor.matmul(
                        ps_t[tile_idx][:, 0, BS : BS + 64], sb_whh[:, 1],
                        _dummy_rhs[:, 0:64], start=False, stop=False,
                        skip_group_check=True,
                    )
            if t > 0:
                dummy(D2_FILL, 0)
            # recurrent mms
            if t > 0 and not no_rc:
                for s in range(NSTREAM):
                    for g in range(4):
                        nc.tensor.matmul(
                            ps_t[s][:, g, 0:BS], sb_whh[:, g], h_tiles[s][:],
                            start=False, stop=True,
                        )
            if t > 0:
                dummy(D1_FILL, 1)
            # merged sigmoid over all 4 gates (g gate pre-scaled by 2 so
            # tanh(x) = 2*sigmoid(2x)-1 costs only a DVE fixup)
            sigs = []
            for s in range(NSTREAM):
                sig = work.tile([128, 4, BS], BF, tag=f"sig{s}")
                nc.scalar.activation(sig[:], ps_t[s][:, :, 0:BS], AF.Sigmoid)
                sigs.append(sig)
            tgs, iis, ffs = [], [], []
            for s in range(NSTREAM):
                sig = sigs[s]
                if t > 0:
                    ff = work.tile([128, BS], BF, tag=f"ff{s}")
                    nc.vector.tensor_tensor(ff[:], sig[:, 1], c_tiles[s][:], ALU.mult)
                    ffs.append(ff)
                tg = work.tile([128, BS], BF, tag=f"tg{s}")
                nc.vector.tensor_scalar(tg[:], sig[:, 3], 2.0, -1.0, ALU.mult, ALU.add)
                tgs.append(tg)
                ii = work.tile([128, BS], BF, tag=f"ii{s}")
                nc.vector.tensor_tensor(ii[:], sig[:, 0], tg[:], ALU.mult)
                iis.append(ii)
            c_news = []
            for s in range(NSTREAM):
                c_new = work.tile([128, BS], BF, tag=f"c{s}")
                if t > 0:
                    nc.vector.tensor_tensor(c_new[:], iis[s][:], ffs[s][:], ALU.add)
                else:
                    nc.vector.tensor_copy(c_new[:], iis[s][:])
                c_news.append(c_new)
            c_tiles = c_news
            h_news = []
            for s in range(NSTREAM):
                tcc = work.tile([128, BS], BF, tag=f"tc{s}")
                nc.scalar.activation(tcc[:], c_tiles[s][:], AF.Tanh)
                h_new = work.tile([128, BS], BF, tag=f"h{s}")
                nc.vector.tensor_tensor(h_new[:], sigs[s][:, 2], tcc[:], ALU.mult)
                h_news.append(h_new)
            h_tiles = h_news

        psL_cm.__exit__(None, None, None)

        # ===================== output head =============================
        psH_cm = tc.tile_pool(name="psH", bufs=1, space="PSUM")
        psH = psH_cm.__enter__()
        out_ps = psH.tile([DOUT, BC], F32, tag="o")
        for s in range(NSTREAM):
            cols = slice(s * BS, (s + 1) * BS)
            nc.tensor.matmul(
                out_ps[:, cols], sb_woh[:], h_tiles[s][:], start=True, stop=False
            )
            nc.tensor.matmul(
                out_ps[:, cols], sb_woc[:], ctxb[:, cols], start=False, stop=True
            )
        for s in range(NSTREAM):
            cols = slice(s * BS, (s + 1) * BS)
            hf = consts.tile([H, BC], F32, tag="dbg_hf")
            nc.vector.tensor_copy(hf[:, cols], h_tiles[s][:])
            cf = consts.tile([H, BC], F32, tag="dbg_cf")
            nc.vector.tensor_copy(cf[:, cols], c_tiles[s][:])
        nc.sync.dma_start(dbg_h[:], hf[:])
        nc.sync.dma_start(dbg_c[:], cf[:])
        nc.sync.dma_start(dbg_ctx[:], ctxn[:])
        nc.sync.dma_start(dbg_qn[:], qn[:])
        out_sb = consts.tile([DOUT, BC], F32, tag="out_sb")
        nc.vector.tensor_scalar(
            out_sb[:], out_ps[:], sb_bo[:, 0:1], None, ALU.add
        )
        nc.sync.dma_start(out[:], out_sb[:])
        psH_cm.__exit__(None, None, None)

    nc.finalize()
    return nc


def _prep_inputs(inputs, W_ih, W_hh, b_ih, b_hh, W_k, b_k, W_o, b_o):
    """Host-side prep: weight layouts, memory bank, per-core shards."""
    f32 = np.float32
    inputs = np.asarray(inputs, f32)
    W_ih = np.asarray(W_ih, f32)
    W_hh = np.asarray(W_hh, f32)
    b = np.asarray(b_ih, f32) + np.asarray(b_hh, f32)
    W_k = np.asarray(W_k, f32)
    b_k = np.asarray(b_k, f32)
    W_o = np.asarray(W_o, f32)
    b_o = np.asarray(b_o, f32)

    # LSTM weights, gate-transposed with bias row, gate order [i,f,o,g]
    wih = np.zeros((DIN + 1, 4, H), f32)
    whh = np.zeros((H, 4, H), f32)
    for j, gs in enumerate(GATE_SRC):
        rows = slice(gs * H, (gs + 1) * H)
        wih[:DIN, j, :] = W_ih[rows].T
        wih[DIN, j, :] = b[rows]
        whh[:, j, :] = W_hh[rows].T
    # fold tanh(x) = 2*sigmoid(2x) - 1 into the g gate (position 3)
    wih[:, 3, :] *= 2.0
    whh[:, 3, :] *= 2.0
    wih = wih.astype(BF16)
    whh = whh.astype(BF16)

    # memory bank from sample 0 (host-side, replicated)
    support = inputs[0, :NSLOTS]
    kp, vp = support[:, :KD], support[:, KD:]
    active = vp.sum(axis=-1) >= NOVELTY
    sk = kp @ W_k.T + b_k
    kn = sk / (np.linalg.norm(sk, axis=-1, keepdims=True) + EPS)
    knt = np.ascontiguousarray(kn.T)  # [DK, NSLOTS]
    maskb = np.where(active, 0.0, -1e5).astype(f32).reshape(NSLOTS, 1)
    vvv = np.ascontiguousarray(vp)  # [NSLOTS, DV]

    wk = np.zeros((KD + 1, DK), f32)
    wk[:KD] = W_k.T
    wk[KD] = b_k

    woh = np.ascontiguousarray(W_o[:, :H].T).astype(BF16)
    woc = np.ascontiguousarray(W_o[:, H:].T).astype(BF16)
    bo = b_o.reshape(DOUT, 1)

    shared = dict(
        wih=wih, whh=whh, wk=wk, knt=knt, maskb=maskb, vv=vvv,
        woh=woh, woc=woc, bo=bo,
    )

    in_maps = []
    for c in range(NCORES):
        shard = inputs[c * BC : (c + 1) * BC]  # [BC, S, DIN]
        xt = shard.transpose(2, 1, 0)  # [DIN, S, BC]
        xaug = np.empty((DIN + 1, S, BC), f32)
        xaug[:DIN] = xt
        xaug[DIN] = 1.0
        xq = np.empty((KD + 1, BC), f32)
        xq[:KD] = shard[:, S - 1, :KD].T
        xq[KD] = 1.0
        m = dict(shared)
        m["xaug"] = xaug.astype(BF16)
        m["xq"] = xq
        in_maps.append(m)
    return in_maps


_CACHED_NC = None


def kernel(inputs, W_ih, W_hh, b_ih, b_hh, W_k, b_k, W_o, b_o,
           _trace=False, _return_raw=False):
    global _CACHED_NC
    in_maps = _prep_inputs(inputs, W_ih, W_hh, b_ih, b_hh, W_k, b_k, W_o, b_o)
    if _CACHED_NC is None:
        _CACHED_NC = build_kernel()
    res = run_bass_kernel_spmd(
        _CACHED_NC, in_maps, core_ids=list(range(NCORES)), trace=_trace
    )
    outs = [np.asarray(res.results[i]["out"], np.float32).T for i in range(NCORES)]
    full = np.concatenate(outs, axis=0)
    if _return_raw:
        return full, res
    return full


# revision 18
# speedup vs baseline: 1.1802x; 1.1802x over previous
"""Trainium2 Bass kernel for nn_AdaptiveModelV3 (LSTM + episodic memory read).

Strategy (hardcoded, per sharding hint): data-parallel over batch on 8
NeuronCores; the tiny memory bank (200x32 keys/vals from sample 0) is
computed host-side and replicated to every core.  No collectives.

Per-core device kernel:
  - LSTM scan over 201 steps in "gate-transposed" layout: partitions =
    128 hidden units of one gate, free dim = batch.  h_t comes out of
    the pointwise stage already transposed for the next step's matmul.
  - Biases are folded into the matmul via a ones-row appended to x
    (contraction K = 65 for the input projection).
  - Gate order in PSUM is [i, f, o, g]: one merged Sigmoid over [i,f,o]
    (FD=768) + one Tanh over g per step per stream.
  - Batch 512/core is split into 2 streams of 256, pipelined so the
    serial pointwise chain of one stream hides under the other's work.
  - Attention read (cosine sim softmax over 200 slots) + output head in
    fp32/bf16, a few microseconds total.
"""

import sys

import numpy as np

try:
    import concourse.bass as bass
except ImportError:  # pragma: no cover
    sys.path.insert(0, "/opt/trn_rl_repo")
    import concourse.bass as bass

import ml_dtypes
from contextlib import ExitStack

import concourse.mybir as mybir
import concourse.tile as tile
from concourse import bacc
from concourse.bass_utils import run_bass_kernel_spmd

BF16 = ml_dtypes.bfloat16

# Problem constants (hardcoded per spec).
B, S, DIN = 4096, 201, 64
H = 128
KD, DK, DV = 32, 32, 32
DOUT = 32
TEMP = 0.1
NOVELTY = 0.5
EPS = 1e-8
NSLOTS = S - 1  # 200

NCORES = 8
BC = B // NCORES  # 512 batch per core
NSTREAM = 2
BS = BC // NSTREAM  # 256 batch per stream
TC = 67  # time chunk (201 = 3 * 67)
NCHUNK = S // TC

# Gate order in reference (PyTorch): [i, f, g, o].  Our PSUM layout is
# [i, f, o, g] so sigmoid covers a contiguous [i,f,o] block.
GATE_SRC = [0, 1, 3, 2]
D1_FILL = 22   # PE filler mms after rc (covers the sigma wait)
D2_FILL = 0    # PE filler mms between ic and rc (covers the h wait)
F32 = mybir.dt.float32
BF = mybir.dt.bfloat16
AF = mybir.ActivationFunctionType
ALU = mybir.AluOpType


def build_kernel(s_steps=S, no_rc=False):
    nc = bacc.Bacc()

    # ---- DRAM parameters (per-core shapes; all cores run the same graph)
    xaug = nc.dram_tensor("xaug", [DIN + 1, S, BC], BF, kind="ExternalInput")
    xq = nc.dram_tensor("xq", [KD + 1, BC], F32, kind="ExternalInput")
    wih = nc.dram_tensor("wih", [DIN + 1, 4, H], BF, kind="ExternalInput")
    whh = nc.dram_tensor("whh", [H, 4, H], BF, kind="ExternalInput")
    wk = nc.dram_tensor("wk", [KD + 1, DK], F32, kind="ExternalInput")
    knt = nc.dram_tensor("knt", [DK, NSLOTS], F32, kind="ExternalInput")
    maskb = nc.dram_tensor("maskb", [NSLOTS, 1], F32, kind="ExternalInput")
    vv = nc.dram_tensor("vv", [NSLOTS, DV], F32, kind="ExternalInput")
    woh = nc.dram_tensor("woh", [H, DOUT], BF, kind="ExternalInput")
    woc = nc.dram_tensor("woc", [DV, DOUT], BF, kind="ExternalInput")
    bo = nc.dram_tensor("bo", [DOUT, 1], F32, kind="ExternalInput")
    out = nc.dram_tensor("out", [DOUT, BC], F32, kind="ExternalOutput")
    dbg_h = nc.dram_tensor("dbg_h", [H, BC], F32, kind="ExternalOutput")
    dbg_c = nc.dram_tensor("dbg_c", [H, BC], F32, kind="ExternalOutput")
    dbg_ctx = nc.dram_tensor("dbg_ctx", [DV, BC], F32, kind="ExternalOutput")
    dbg_qn = nc.dram_tensor("dbg_qn", [DK, BC], F32, kind="ExternalOutput")

    with tile.TileContext(nc) as tc, ExitStack() as ctx:
        consts = ctx.enter_context(tc.tile_pool(name="consts", bufs=1))
        xpool = ctx.enter_context(tc.tile_pool(name="xp", bufs=2))
        work = ctx.enter_context(tc.tile_pool(name="work", bufs=2))

        # ---- weights / consts to SBUF
        sb_wih = consts.tile([DIN + 1, 4, H], BF, tag="wih")
        nc.sync.dma_start(sb_wih[:], wih[:])
        sb_whh = consts.tile([H, 4, H], BF, tag="whh")
        nc.sync.dma_start(sb_whh[:], whh[:])
        sb_xq = consts.tile([KD + 1, BC], F32, tag="xq")
        nc.sync.dma_start(sb_xq[:], xq[:])
        sb_wk = consts.tile([KD + 1, DK], F32, tag="wk")
        nc.sync.dma_start(sb_wk[:], wk[:])
        sb_knt = consts.tile([DK, NSLOTS], F32, tag="knt")
        nc.sync.dma_start(sb_knt[:], knt[:])
        sb_mb1 = consts.tile([128, 1], F32, tag="mb1")
        nc.sync.dma_start(sb_mb1[:], maskb[0:128])
        sb_mb2 = consts.tile([NSLOTS - 128, 1], F32, tag="mb2")
        nc.sync.dma_start(sb_mb2[:], maskb[128:NSLOTS])
        sb_v1 = consts.tile([128, DV], F32, tag="v1")
        nc.sync.dma_start(sb_v1[:], vv[0:128])
        sb_v2 = consts.tile([NSLOTS - 128, DV], F32, tag="v2")
        nc.sync.dma_start(sb_v2[:], vv[128:NSLOTS])
        sb_woh = consts.tile([H, DOUT], BF, tag="woh")
        nc.sync.dma_start(sb_woh[:], woh[:])
        sb_woc = consts.tile([DV, DOUT], BF, tag="woc")
        nc.sync.dma_start(sb_woc[:], woc[:])
        sb_bo = consts.tile([DOUT, 1], F32, tag="bo")
        nc.sync.dma_start(sb_bo[:], bo[:])

        ones_q = consts.tile([DK, DK], F32, tag="ones_q")
        nc.vector.memset(ones_q[:], 1.0)
        ones_s1 = consts.tile([128, DK], F32, tag="ones_s1")
        nc.vector.memset(ones_s1[:], 1.0)
        ones_s2 = consts.tile([NSLOTS - 128, DK], F32, tag="ones_s2")
        nc.vector.memset(ones_s2[:], 1.0)

        # ================= attention read (independent of the LSTM) ====
        psA_cm = tc.tile_pool(name="psA", bufs=1, space="PSUM")
        psA = psA_cm.__enter__()
        # qT = Wk @ [x_q; 1]  -> [DK, BC] in PSUM
        q_ps = psA.tile([DK, BC], F32, tag="q")
        nc.tensor.matmul(q_ps[:], sb_wk[:], sb_xq[:], start=True, stop=True)
        # copy to SBUF (Copy is in every ACT table set)
        qc = consts.tile([DK, BC], F32, tag="qc")
        nc.scalar.copy(qc[:], q_ps[:])
        # squared + column sums (replicated to all DK partitions via ones)
        q2 = consts.tile([DK, BC], F32, tag="q2")
        nc.vector.tensor_tensor(q2[:], qc[:], qc[:], ALU.mult)
        n_ps = psA.tile([DK, BC], F32, tag="n")
        nc.tensor.matmul(n_ps[:], ones_q[:], q2[:], start=True, stop=True)
        # s = n^(-1/2) = exp(-0.5 * ln(n))   (||q|| >> eps, so eps ignored)
        lnn = consts.tile([DK, BC], F32, tag="lnn")
        nc.scalar.activation(lnn[:], n_ps[:], AF.Ln)
        s32 = consts.tile([DK, BC], F32, tag="s32")
        nc.scalar.activation(s32[:], lnn[:], AF.Exp, scale=-0.5)
        qn = consts.tile([DK, BC], F32, tag="qn")
        nc.vector.tensor_tensor(qn[:], qc[:], s32[:], ALU.mult)

        # logits chunks: L = kn @ qn  ([slots, BC])
        l1_ps = psA.tile([128, BC], F32, tag="l1")
        nc.tensor.matmul(l1_ps[:], sb_knt[:, 0:128], qn[:], start=True, stop=True)
        l2_ps = psA.tile([NSLOTS - 128, BC], F32, tag="l2")
        nc.tensor.matmul(l2_ps[:], sb_knt[:, 128:NSLOTS], qn[:], start=True, stop=True)
        e1 = consts.tile([128, BC], F32, tag="e1")
        nc.scalar.activation(e1[:], l1_ps[:], AF.Exp, bias=sb_mb1[:], scale=1.0 / TEMP)
        e2 = consts.tile([NSLOTS - 128, BC], F32, tag="e2")
        nc.scalar.activation(e2[:], l2_ps[:], AF.Exp, bias=sb_mb2[:], scale=1.0 / TEMP)

        # denominator, replicated over DK partitions; r = 1/S
        s_ps = psA.tile([DK, BC], F32, tag="s")
        nc.tensor.matmul(s_ps[:], ones_s1[:], e1[:], start=True, stop=False)
        nc.tensor.matmul(s_ps[:], ones_s2[:], e2[:], start=False, stop=True)
        lns = consts.tile([DK, BC], F32, tag="lns")
        nc.scalar.activation(lns[:], s_ps[:], AF.Ln)
        rs = consts.tile([DK, BC], F32, tag="rs")
        nc.scalar.activation(rs[:], lns[:], AF.Exp, scale=-1.0)

        # ctxT = V^T @ E  -> [DV, BC]; then normalize and cast to bf16
        ctx_ps = psA.tile([DV, BC], F32, tag="c")
        nc.tensor.matmul(ctx_ps[:], sb_v1[:], e1[:], start=True, stop=False)
        nc.tensor.matmul(ctx_ps[:], sb_v2[:], e2[:], start=False, stop=True)
        ctxn = consts.tile([DV, BC], F32, tag="ctxn")
        nc.vector.tensor_tensor(ctxn[:], ctx_ps[:], rs[:], ALU.mult)
        ctxb = consts.tile([DV, BC], BF, tag="ctxb")
        nc.vector.tensor_copy(ctxb[:], ctxn[:])

        psA_cm.__exit__(None, None, None)

        # ========================= LSTM scan ===========================
        psL_cm = tc.tile_pool(name="psL", bufs=1, space="PSUM")
        psL = psL_cm.__enter__()
        h_tiles = [None, None]
        c_tiles = [None, None]
        xc_tiles = {}

        for ci in range(NCHUNK):
            xc = xpool.tile([DIN + 1, TC, BC], BF, tag="xc")
            nc.sync.dma_start(xc[:], xaug[:, ci * TC : (ci + 1) * TC, :])
            xc_tiles[ci] = xc

        _dummy_rhs = sb_whh[:, 1:3].rearrange("k a b -> k (a b)")
        # psum per stream: [128, 4, 512] fp32, ONE BANK PER GATE (cols
        # 0:BS used).  start=True clears has_written at bank granularity,
        # so bank isolation lets ic mms prefetch in any order before the
        # h-dependent rc mms.  bufs=1: 4 banks x 2 streams = all of PSUM;
        # the only psum reader is the merged sigmoid, so step t+1's ic
        # mms just wait on sigma(t).
        for t in range(s_steps):
            xc = xc_tiles[t // TC]
            ti = t % TC
            ps_t = []
            for s in range(NSTREAM):
                ps = psL.tile([128, 4, 512], F32, tag=f"lstm{s}")
                ps_t.append(ps)
            # input-projection mms: prefetchable (no h dependency)
            for g in range(4):
                for s in range(NSTREAM):
                    xs = xc[:, ti, s * BS : (s + 1) * BS]
                    nc.tensor.matmul(
                        ps_t[s][:, g, 0:BS], sb_wih[:, g], xs,
                        start=True, stop=(t == 0),
                    )
            if t == 0:
                # PE warm-up: ~48 back-to-back matmuls into the unused
                # scratch half of the step-0 psum banks.  One contiguous
                # >3.4us burst flips the HAM clock gate to 2.4GHz; the
                # per-step gaps afterwards are too short to re-throttle.
                for w in range(48):
                    nc.tensor.matmul(
                        ps_t[0][:, 0, BS : BS + 128], sb_whh[:, 1],
                        sb_whh[:, 2], start=False, stop=False,
                        skip_group_check=True,
                    )
            # PE filler: keep duty near 100% so HAM holds the 2.4GHz clock.
            # Dummies write the unused scratch half of the step's psum banks.
            def dummy(n, tile_idx=0):
                for _ in range(n):
                    nc.tensor.matmul(
                        ps_t[tile_idx][:, 0, BS : BS + 64], sb_whh[:, 1],
                        _dummy_rhs[:, 0:64], start=False, stop=False,
                        skip_group_check=True,
                    )
            if t > 0:
                dummy(D2_FILL, 0)
            # recurrent mms
            if t > 0 and not no_rc:
                for s in range(NSTREAM):
                    for g in range(4):
                        nc.tensor.matmul(
                            ps_t[s][:, g, 0:BS], sb_whh[:, g], h_tiles[s][:],
                            start=False, stop=True,
                        )
            if t > 0:
                dummy(D1_FILL, 1)
            # merged sigmoid over all 4 gates (g gate pre-scaled by 2 so
            # tanh(x) = 2*sigmoid(2x)-1 costs only a DVE fixup)
            sigs = []
            for s in range(NSTREAM):
                sig = work.tile([128, 4, BS], BF, tag=f"sig{s}")
                nc.scalar.activation(sig[:], ps_t[s][:, :, 0:BS], AF.Sigmoid)
                sigs.append(sig)
            tgs, iis, ffs = [], [], []
            for s in range(NSTREAM):
                sig = sigs[s]
                if t > 0:
                    ff = work.tile([128, BS], BF, tag=f"ff{s}")
                    nc.vector.tensor_tensor(ff[:], sig[:, 1], c_tiles[s][:], ALU.mult)
                    ffs.append(ff)
                tg = work.tile([128, BS], BF, tag=f"tg{s}")
                nc.vector.tensor_scalar(tg[:], sig[:, 3], 2.0, -1.0, ALU.mult, ALU.add)
                tgs.append(tg)
                ii = work.tile([128, BS], BF, tag=f"ii{s}")
                nc.vector.tensor_tensor(ii[:], sig[:, 0], tg[:], ALU.mult)
                iis.append(ii)
            c_news = []
            for s in range(NSTREAM):
                c_new = work.tile([128, BS], BF, tag=f"c{s}")
                if t > 0:
                    nc.vector.tensor_tensor(c_new[:], iis[s][:], ffs[s][:], ALU.add)
                else:
                    nc.vector.tensor_copy(c_new[:], iis[s][:])
                c_news.append(c_new)
            c_tiles = c_news
            h_news = []
            for s in range(NSTREAM):
                tcc = work.tile([128, BS], BF, tag=f"tc{s}")
                nc.scalar.activation(tcc[:], c_tiles[s][:], AF.Tanh)
                h_new = work.tile([128, BS], BF, tag=f"h{s}")
                nc.vector.tensor_tensor(h_new[:], sigs[s][:, 2], tcc[:], ALU.mult)
                h_news.append(h_new)
            h_tiles = h_news

        psL_cm.__exit__(None, None, None)

        # ===================== output head =============================
        psH_cm = tc.tile_pool(name="psH", bufs=1, space="PSUM")
        psH = psH_cm.__enter__()
        out_ps = psH.tile([DOUT, BC], F32, tag="o")
        for s in range(NSTREAM):
            cols = slice(s * BS, (s + 1) * BS)
            nc.tensor.matmul(
                out_ps[:, cols], sb_woh[:], h_tiles[s][:], start=True, stop=False
            )
            nc.tensor.matmul(
                out_ps[:, cols], sb_woc[:], ctxb[:, cols], start=False, stop=True
            )
        for s in range(NSTREAM):
            cols = slice(s * BS, (s + 1) * BS)
            hf = consts.tile([H, BC], F32, tag="dbg_hf")
            nc.vector.tensor_copy(hf[:, cols], h_tiles[s][:])
            cf = consts.tile([H, BC], F32, tag="dbg_cf")
            nc.vector.tensor_copy(cf[:, cols], c_tiles[s][:])
        nc.sync.dma_start(dbg_h[:], hf[:])
        nc.sync.dma_start(dbg_c[:], cf[:])
        nc.sync.dma_start(dbg_ctx[:], ctxn[:])
        nc.sync.dma_start(dbg_qn[:], qn[:])
        out_sb = consts.tile([DOUT, BC], F32, tag="out_sb")
        nc.vector.tensor_scalar(
            out_sb[:], out_ps[:], sb_bo[:, 0:1], None, ALU.add
        )
        nc.sync.dma_start(out[:], out_sb[:])
        psH_cm.__exit__(None, None, None)

    nc.finalize()
    return nc


def _prep_inputs(inputs, W_ih, W_hh, b_ih, b_hh, W_k, b_k, W_o, b_o):
    """Host-side prep: weight layouts, memory bank, per-core shards."""
    f32 = np.float32
    inputs = np.asarray(inputs, f32)
    W_ih = np.asarray(W_ih, f32)
    W_hh = np.asarray(W_hh, f32)
    b = np.asarray(b_ih, f32) + np.asarray(b_hh, f32)
    W_k = np.asarray(W_k, f32)
    b_k = np.asarray(b_k, f32)
    W_o = np.asarray(W_o, f32)
    b_o = np.asarray(b_o, f32)

    # LSTM weights, gate-transposed with bias row, gate order [i,f,o,g]
    wih = np.zeros((DIN + 1, 4, H), f32)
    whh = np.zeros((H, 4, H), f32)
    for j, gs in enumerate(GATE_SRC):
        rows = slice(gs * H, (gs + 1) * H)
        wih[:DIN, j, :] = W_ih[rows].T
        wih[DIN, j, :] = b[rows]
        whh[:, j, :] = W_hh[rows].T
    # fold tanh(x) = 2*sigmoid(2x) - 1 into the g gate (position 3)
    wih[:, 3, :] *= 2.0
    whh[:, 3, :] *= 2.0
    wih = wih.astype(BF16)
    whh = whh.astype(BF16)

    # memory bank from sample 0 (host-side, replicated)
    support = inputs[0, :NSLOTS]
    kp, vp = support[:, :KD], support[:, KD:]
    active = vp.sum(axis=-1) >= NOVELTY
    sk = kp @ W_k.T + b_k
    kn = sk / (np.linalg.norm(sk, axis=-1, keepdims=True) + EPS)
    knt = np.ascontiguousarray(kn.T)  # [DK, NSLOTS]
    maskb = np.where(active, 0.0, -1e5).astype(f32).reshape(NSLOTS, 1)
    vvv = np.ascontiguousarray(vp)  # [NSLOTS, DV]

    wk = np.zeros((KD + 1, DK), f32)
    wk[:KD] = W_k.T
    wk[KD] = b_k

    woh = np.ascontiguousarray(W_o[:, :H].T).astype(BF16)
    woc = np.ascontiguousarray(W_o[:, H:].T).astype(BF16)
    bo = b_o.reshape(DOUT, 1)

    shared = dict(
        wih=wih, whh=whh, wk=wk, knt=knt, maskb=maskb, vv=vvv,
        woh=woh, woc=woc, bo=bo,
    )

    in_maps = []
    for c in range(NCORES):
        shard = inputs[c * BC : (c + 1) * BC]  # [BC, S, DIN]
        xt = shard.transpose(2, 1, 0)  # [DIN, S, BC]
        xaug = np.empty((DIN + 1, S, BC), f32)
        xaug[:DIN] = xt
        xaug[DIN] = 1.0
        xq = np.empty((KD + 1, BC), f32)
        xq[:KD] = shard[:, S - 1, :KD].T
        xq[KD] = 1.0
        m = dict(shared)
        m["xaug"] = xaug.astype(BF16)
        m["xq"] = xq
        in_maps.append(m)
    return in_maps


_CACHED_NC = None


def kernel(inputs, W_ih, W_hh, b_ih, b_hh, W_k, b_k, W_o, b_o,
           _trace=False, _return_raw=False):
    global _CACHED_NC
    in_maps = _prep_inputs(inputs, W_ih, W_hh, b_ih, b_hh, W_k, b_k, W_o, b_o)
    if _CACHED_NC is None:
        _CACHED_NC = build_kernel()
    res = run_bass_kernel_spmd(
        _CACHED_NC, in_maps, core_ids=list(range(NCORES)), trace=_trace
    )
    outs = [np.asarray(res.results[i]["out"], np.float32).T for i in range(NCORES)]
    full = np.concatenate(outs, axis=0)
    if _return_raw:
        return full, res
    return full
